# revision 5
# baseline (speedup 1.0000x reference)
"""Trainium2 Bass kernel for nn_DecoderModel_74131135529564 (gnn_message_passing).

Model (per reference):
  x_graph = segment_mean(x, batch)                     # [G, D]
  g_out   = (relu(x_graph @ Wgs[b] + bgs[b]) @ Wgh[b] + bgh[b])   b = dataset_name[g]
  n_out   = (relu(x @ Wn1[b] + bn1[b]) @ Wn2[b] + bn2[b])         b = dataset_name[batch[n]]
  returns (g_out[:, :50], n_out[:, :3], g_out[:, 50:]**2, n_out[:, 3:]**2)

Strategy: data-parallel over graphs across 8 cores. Host sorts graphs by
branch and deals them round-robin so every core gets a balanced mix, then
pads each branch section to a multiple of 16 graphs (512 nodes = one
compute block). Every 512-node block is branch-uniform, so weight
selection is static in the program. x is cast to bf16 on host and loaded
with the xbar DMA transpose so the node MLP contraction dim (D) lands on
partitions. Graph pooling is computed with dense tree-folds on the vector
engine directly from the transposed layout; the graph head runs in
float32r at full PE speed.
"""

import math

import ml_dtypes
import numpy as np

import concourse.bacc as bacc
import concourse.bass as bass
import concourse.mybir as mybir
import concourse.tile as tile
from concourse import bass_utils

G = 4096
NPG = 32
N = G * NPG
D = 256
SH = 256
HDG = 50
HDN = 3
NB = 2
NCORES = 8

f32 = mybir.dt.float32
f32r = mybir.dt.float32r
bf16 = mybir.dt.bfloat16
AF = mybir.ActivationFunctionType

_programs: dict = {}


def _g_tiles(S0, S1):
    """Graph-head M-tiles: list of (section, col0_within_section, msz)."""
    tiles = []
    for sec, S in ((0, S0), (1, S1)):
        gsz = 16 * S
        for mt in range(0, gsz, 128):
            tiles.append((sec, mt, min(128, gsz - mt)))
    return tiles


def _build_program(S0, S1, repeat=1):
    NBLK = S0 + S1          # 512-node blocks per core
    GP = 16 * NBLK          # padded graphs per core
    NP = 512 * NBLK         # padded nodes per core
    gtiles = _g_tiles(S0, S1)
    NTG = len(gtiles)

    nc = bacc.Bacc("TRN2", target_bir_lowering=False, debug=False,
                   num_devices=NCORES)

    xk_d = [nc.dram_tensor(f"xk{k}", [NP, 128], bf16, kind="ExternalInput").ap()
            for k in range(2)]
    wn1_d = nc.dram_tensor("wn1", [NB, D, SH], bf16, kind="ExternalInput").ap()
    wn2_d = nc.dram_tensor("wn2", [NB, SH, 8], bf16, kind="ExternalInput").ap()
    wgs_d = nc.dram_tensor("wgs", [NB, D, SH], f32r, kind="ExternalInput").ap()
    wgh_d = nc.dram_tensor("wgh", [NB, SH, 100], bf16, kind="ExternalInput").ap()
    bn1_d = nc.dram_tensor("bn1", [NB, SH], f32, kind="ExternalInput").ap()
    bgs_d = nc.dram_tensor("bgs", [NB, SH], f32, kind="ExternalInput").ap()
    bn2r_d = nc.dram_tensor("bn2r", [NB, 128, 24], f32, kind="ExternalInput").ap()
    bghr_d = nc.dram_tensor("bghr", [NB, 128, 100], f32, kind="ExternalInput").ap()
    nbuf_d = nc.dram_tensor("nbuf", [128, NBLK * 24], f32, kind="ExternalOutput").ap()
    gbuf_d = nc.dram_tensor("gbuf", [128, NTG * 100], f32, kind="ExternalOutput").ap()

    # xT load slices: whole blocks, ~5 blocks (1.3 MiB) per transpose DMA
    slices = []
    b0 = 0
    while b0 < NBLK:
        sb = min(5, NBLK - b0)
        slices.append((b0, sb))
        b0 += sb
    blk_slice = {}
    for si, (sb0, sbn) in enumerate(slices):
        for b in range(sb0, sb0 + sbn):
            blk_slice[b] = (si, b - sb0)

    with tile.TileContext(nc) as tc:
        with (
            tc.tile_pool(name="wp", bufs=1) as wp,
            tc.tile_pool(name="xp", bufs=1) as xp,
            tc.tile_pool(name="fp", bufs=1) as fp,
            tc.tile_pool(name="hp", bufs=3) as hp,
            tc.tile_pool(name="op", bufs=1) as op,
            tc.tile_pool(name="psm", bufs=4, space="PSUM") as psm,
            tc.tile_pool(name="pss", bufs=3, space="PSUM") as pss,
        ):
            # ---------------- weights / biases (resident) ----------------
            wn1_t = [[wp.tile([128, SH], bf16, name=f"wn1_{b}_{k}")
                      for k in range(2)] for b in range(2)]
            wn2_t = [[wp.tile([128, 8], bf16, name=f"wn2_{b}_{k}")
                      for k in range(2)] for b in range(2)]
            wgs_t = [[wp.tile([128, SH], f32r, name=f"wgs_{b}_{k}")
                      for k in range(2)] for b in range(2)]
            wgh_t = [[wp.tile([128, 100], bf16, name=f"wgh_{b}_{k}")
                      for k in range(2)] for b in range(2)]
            bn1_t = [[wp.tile([128, 1], f32, name=f"bn1_{b}_{m}")
                      for m in range(2)] for b in range(2)]
            bgs_t = [[wp.tile([128, 1], f32, name=f"bgs_{b}_{m}")
                      for m in range(2)] for b in range(2)]
            bn2r_t = [wp.tile([128, 24], f32, name=f"bn2r_{b}") for b in range(2)]
            bghr_t = [wp.tile([128, 100], f32, name=f"bghr_{b}") for b in range(2)]
            for b in range(2):
                for k in range(2):
                    nc.sync.dma_start(out=wn1_t[b][k],
                                      in_=wn1_d[b, k * 128:(k + 1) * 128, :])
                    nc.sync.dma_start(out=wn2_t[b][k],
                                      in_=wn2_d[b, k * 128:(k + 1) * 128, :])
                    nc.sync.dma_start(out=wgs_t[b][k],
                                      in_=wgs_d[b, k * 128:(k + 1) * 128, :])
                    nc.sync.dma_start(out=wgh_t[b][k],
                                      in_=wgh_d[b, k * 128:(k + 1) * 128, :])
                for m in range(2):
                    nc.sync.dma_start(
                        out=bn1_t[b][m],
                        in_=bn1_d[b, m * 128:(m + 1) * 128].rearrange("(a o) -> a o", o=1))
                    nc.sync.dma_start(
                        out=bgs_t[b][m],
                        in_=bgs_d[b, m * 128:(m + 1) * 128].rearrange("(a o) -> a o", o=1))
                nc.sync.dma_start(out=bn2r_t[b], in_=bn2r_d[b])
                nc.sync.dma_start(out=bghr_t[b], in_=bghr_d[b])

            for _rep in range(repeat):
                # ---------------- x load (transposed, sliced) ----------------
                xT = [[xp.tile([128, 512 * sbn], bf16, name=f"xT_{k}_{si}", tag=f"xT_{k}_{si}")
                       for si, (sb0, sbn) in enumerate(slices)] for k in range(2)]
                for k in range(2):
                    for si, (sb0, sbn) in enumerate(slices):
                        nc.sync.dma_start_transpose(
                            out=xT[k][si],
                            in_=xk_d[k][sb0 * 512:(sb0 + sbn) * 512, :])

                # ---------------- pooling folds (per slice) ----------------
                xgT = [fp.tile([128, GP], f32r, name=f"xgT_{k}", tag=f"xgT_{k}")
                       for k in range(2)]
                for k in range(2):
                    for si, (sb0, sbn) in enumerate(slices):
                        g0, gn = sb0 * 16, sbn * 16
                        v = xT[k][si].rearrange("p (g s) -> p g s", s=32)
                        f1 = fp.tile([128, gn, 16], bf16, name="f1", tag="f1")
                        nc.vector.tensor_add(f1, v[:, :, 0:16], v[:, :, 16:32])
                        f2 = fp.tile([128, gn, 8], bf16, name="f2", tag="f2")
                        nc.vector.tensor_add(f2, f1[:, :, 0:8], f1[:, :, 8:16])
                        f3 = fp.tile([128, gn, 4], f32, name="f3", tag="f3")
                        nc.vector.tensor_add(f3, f2[:, :, 0:4], f2[:, :, 4:8])
                        f4 = fp.tile([128, gn, 2], f32, name="f4", tag="f4")
                        nc.vector.tensor_add(f4, f3[:, :, 0:2], f3[:, :, 2:4])
                        nc.vector.tensor_add(
                            xgT[k][:, g0:g0 + gn].rearrange("p (g o) -> p g o", o=1),
                            f4[:, :, 0:1], f4[:, :, 1:2])

                # ---------------- node path ----------------
                nbuf = op.tile([128, NBLK * 24], f32, name="nbuf", tag="nbuf")
                for blk in range(NBLK):
                    b = 0 if blk < S0 else 1
                    si, lb = blk_slice[blk]
                    xv = xT[0][si][:, lb * 512:(lb + 1) * 512]
                    xv1 = xT[1][si][:, lb * 512:(lb + 1) * 512]
                    hnT = []
                    for m in range(2):
                        ps = psm.tile([128, 512], f32, name="mm", tag="mm")
                        nc.tensor.matmul(ps, wn1_t[b][0][:, m * 128:(m + 1) * 128],
                                         xv, start=True, stop=False)
                        nc.tensor.matmul(ps, wn1_t[b][1][:, m * 128:(m + 1) * 128],
                                         xv1, start=False, stop=True)
                        h = hp.tile([128, 512], bf16, name="hnT", tag="hnT")
                        if (blk * 2 + m) % 8 < 5:
                            nc.scalar.activation(h, ps, AF.Relu, bias=bn1_t[b][m])
                        else:
                            nc.vector.tensor_scalar(
                                h, ps, bn1_t[b][m], 0.0,
                                op0=mybir.AluOpType.add, op1=mybir.AluOpType.max)
                        hnT.append(h)
                    o2 = pss.tile([128, 24], f32, name="o2", tag="small")
                    for j in range(4):
                        oj = o2[:, j * 6:(j + 1) * 6]
                        nc.tensor.matmul(oj, hnT[0][:, j * 128:(j + 1) * 128],
                                         wn2_t[b][0][:, 0:6], start=True, stop=False)
                        nc.tensor.matmul(oj, hnT[1][:, j * 128:(j + 1) * 128],
                                         wn2_t[b][1][:, 0:6], start=False, stop=True)
                    nc.vector.tensor_add(nbuf[:, blk * 24:(blk + 1) * 24], o2,
                                         bn2r_t[b])
                nv = nbuf.rearrange("p (t c) -> p t c", c=6)[:, :, 3:6]
                nc.scalar.activation(nv, nv, AF.Square)
                nc.sync.dma_start(out=nbuf_d, in_=nbuf)

                # ---------------- graph path ----------------
                gbuf = op.tile([128, NTG * 100], f32, name="gbuf", tag="gbuf")
                nc.vector.memset(gbuf, 0.0)
                hgT_sec: dict = {}
                for sec, S in ((0, S0), (1, S1)):
                    gsz = 16 * S
                    gs0 = 0 if sec == 0 else 16 * S0
                    hg = []
                    for m in range(2):
                        ps = psm.tile([128, 512], f32, name="mm", tag="mm")
                        nc.tensor.matmul(ps[:, :gsz],
                                         wgs_t[sec][0][:, m * 128:(m + 1) * 128],
                                         xgT[0][:, gs0:gs0 + gsz],
                                         start=True, stop=False)
                        nc.tensor.matmul(ps[:, :gsz],
                                         wgs_t[sec][1][:, m * 128:(m + 1) * 128],
                                         xgT[1][:, gs0:gs0 + gsz],
                                         start=False, stop=True)
                        h = hp.tile([128, 512], bf16, name="hgT", tag="hgT", bufs=4)
                        nc.scalar.activation(h[:, :gsz], ps[:, :gsz], AF.Relu,
                                             bias=bgs_t[sec][m])
                        hg.append(h)
                    hgT_sec[sec] = hg
                for ti, (sec, mt, msz) in enumerate(gtiles):
                    og = pss.tile([128, 100], f32, name="og", tag="small")
                    nc.tensor.matmul(og[:msz, :],
                                     hgT_sec[sec][0][:, mt:mt + msz],
                                     wgh_t[sec][0], start=True, stop=False)
                    nc.tensor.matmul(og[:msz, :],
                                     hgT_sec[sec][1][:, mt:mt + msz],
                                     wgh_t[sec][1], start=False, stop=True)
                    nc.vector.tensor_add(gbuf[:msz, ti * 100:(ti + 1) * 100],
                                         og[:msz, :], bghr_t[sec][:msz, :])
                gv = gbuf.rearrange("p (t c) -> p t c", c=100)[:, :, 50:100]
                nc.scalar.activation(gv, gv, AF.Square)
                nc.sync.dma_start(out=gbuf_d, in_=gbuf)

    nc.compile()
    return nc


def _prepare(x, dataset_name, Wgs, bgs, Wgh, bgh, Wn1, bn1, Wn2, bn2):
    ds = np.asarray(dataset_name)
    assert ds.shape == (G,)
    g0 = np.where(ds == 0)[0]
    g1 = np.where(ds != 0)[0]
    per0 = [g0[c::NCORES] for c in range(NCORES)]
    per1 = [g1[c::NCORES] for c in range(NCORES)]
    S0 = max(1, math.ceil(max(len(p) for p in per0) / 16))
    S1 = max(1, math.ceil(max(len(p) for p in per1) / 16))
    NBLK = S0 + S1
    GP = 16 * NBLK

    # slot map: per core, padded graph slots -> original graph id (-1 = pad)
    slot = np.zeros((NCORES, GP), dtype=np.int64)
    valid = np.zeros((NCORES, GP), dtype=bool)
    for c in range(NCORES):
        s0 = np.full(16 * S0, per0[c][0] if len(per0[c]) else 0, dtype=np.int64)
        s0[: len(per0[c])] = per0[c]
        s1 = np.full(16 * S1, per1[c][0] if len(per1[c]) else 0, dtype=np.int64)
        s1[: len(per1[c])] = per1[c]
        slot[c] = np.concatenate([s0, s1])
        valid[c, : len(per0[c])] = True
        valid[c, 16 * S0 : 16 * S0 + len(per1[c])] = True

    xb = np.asarray(x).astype(ml_dtypes.bfloat16).reshape(G, NPG, D)
    w = {
        "wn1": np.asarray(Wn1).astype(ml_dtypes.bfloat16),
        "wgs": np.ascontiguousarray(np.asarray(Wgs, np.float32) / NPG),
        "wgh": np.asarray(Wgh).astype(ml_dtypes.bfloat16),
        "bn1": np.asarray(bn1, np.float32),
        "bgs": np.asarray(bgs, np.float32),
    }
    wn2 = np.zeros((NB, SH, 8), ml_dtypes.bfloat16)
    wn2[:, :, :6] = np.asarray(Wn2).astype(ml_dtypes.bfloat16)
    w["wn2"] = wn2
    bn2r = np.broadcast_to(
        np.tile(np.asarray(bn2, np.float32), (1, 4))[:, None, :], (NB, 128, 24))
    w["bn2r"] = np.ascontiguousarray(bn2r)
    bghr = np.broadcast_to(np.asarray(bgh, np.float32)[:, None, :], (NB, 128, 100))
    w["bghr"] = np.ascontiguousarray(bghr)

    in_maps = []
    for c in range(NCORES):
        xc = xb[slot[c]].reshape(GP * NPG, D)
        m = {f"xk{k}": np.ascontiguousarray(xc[:, k * 128:(k + 1) * 128])
             for k in range(2)}
        m.update(w)
        in_maps.append(m)
    return S0, S1, slot, valid, in_maps


def _unpack(results, S0, S1, slot, valid):
    NBLK = S0 + S1
    GP = 16 * NBLK
    NPc = 512 * NBLK
    gtiles = _g_tiles(S0, S1)

    n_head = np.zeros((N, HDN), np.float32)
    n_var = np.zeros((N, HDN), np.float32)
    g_head = np.zeros((G, HDG), np.float32)
    g_var = np.zeros((G, HDG), np.float32)

    for c in range(NCORES):
        narr = results[c]["nbuf"].reshape(128, NBLK, 4, 6)
        narr = narr.transpose(1, 2, 0, 3).reshape(NPc, 6)
        v = valid[c]
        gids = slot[c][v]
        nodes = (gids[:, None] * NPG + np.arange(NPG)[None, :]).reshape(-1)
        src = (np.where(v)[0][:, None] * NPG + np.arange(NPG)[None, :]).reshape(-1)
        n_head[nodes] = narr[src, :3]
        n_var[nodes] = narr[src, 3:]

        garr = np.zeros((GP, 100), np.float32)
        gb = results[c]["gbuf"]
        for ti, (sec, mt, msz) in enumerate(gtiles):
            gs0 = 0 if sec == 0 else 16 * S0
            garr[gs0 + mt : gs0 + mt + msz] = gb[:msz, ti * 100:(ti + 1) * 100]
        g_head[gids] = garr[np.where(v)[0], :50]
        g_var[gids] = garr[np.where(v)[0], 50:]
    return g_head, n_head, g_var, n_var


def kernel(x, dataset_name, batch, Wgs, bgs, Wgh, bgh, Wn1, bn1, Wn2, bn2):
    S0, S1, slot, valid, in_maps = _prepare(
        x, dataset_name, Wgs, bgs, Wgh, bgh, Wn1, bn1, Wn2, bn2)
    key = (S0, S1)
    if key not in _programs:
        _programs[key] = _build_program(S0, S1)
    nc = _programs[key]
    res = bass_utils.run_bass_kernel_spmd(nc, in_maps,
                                          core_ids=list(range(NCORES)))
    return _unpack(res.results, S0, S1, slot, valid)


# revision 18
# speedup vs baseline: 15112.2810x; 15112.2810x over previous
"""Trainium2 Bass kernel for nn_DecoderModel_74131135529564 (gnn_message_passing).

Model (per reference):
  x_graph = segment_mean(x, batch)                              # [G, D]
  g_out = relu(x_graph @ Wgs[b] + bgs[b]) @ Wgh[b] + bgh[b]     b = dataset_name[g]
  n_out = relu(x @ Wn1[b] + bn1[b]) @ Wn2[b] + bn2[b]           b = dataset_name[batch[n]]
  returns (g_out[:, :50], n_out[:, :3], g_out[:, 50:]**2, n_out[:, 3:]**2)

Strategy: data-parallel over graphs on 8 cores. Host deals graphs of each
branch round-robin to cores (balanced mix), pads each branch section to a
multiple of 16 graphs (512 nodes = one block), so every block is
branch-uniform and weight selection is static. x is cast to bf16 on host
and loaded via xbar DMA-transpose so the contraction dim D lands on
partitions. Pooling = tree folds on DVE (bf16 2x mode for the wide
levels). Node L2 uses weights-stationary matmuls col-packed 4 blocks per
PSUM bank via tile_position; head copy and var square+bias are fused into
two activation copies. Graph head runs transposed (out dims on
partitions) in float32r / bf16.
"""

import math

import ml_dtypes
import numpy as np

import concourse.bacc as bacc
import concourse.bass as bass
import concourse.mybir as mybir
import concourse.tile as tile
from concourse import bass_utils

G = 4096
NPG = 32
N = G * NPG
D = 256
SH = 256
HDG = 50
HDN = 3
NB = 2
NCORES = 8

f32 = mybir.dt.float32
f32r = mybir.dt.float32r
bf16 = mybir.dt.bfloat16
AF = mybir.ActivationFunctionType
ALU = mybir.AluOpType

_programs: dict = {}

# packed bf16 weight blob column layout (per [128, _] tile):
#   wn1[b][k] -> cols [ (b*2+k)*256 , +256 )          (4 x 256 = 1024)
#   wn2[b][k] -> cols [ 1024 + (b*2+k)*32 , +32 )     (4 x 32  = 128)  cols 6..31 zero
#   wgh[b][k] -> cols [ 1152 + (b*2+k)*128 , +128 )   (4 x 128; head 0:50, var 64:114)
WBF_COLS = 1664
# packed f32 blob: bn1[b][m] -> col b*2+m (4), bgs[b][m] -> 4+b*2+m (4),
#   bn2col[b] -> 8+b (2), bghT[b] -> 10+b (2)
WF32_COLS = 12
# wgs f32r blob: wgs[b][k] -> cols [(b*2+k)*256, +256)
WGS_COLS = 1024


def _build_program(S0, S1, repeat=1, relu_dve_mod=4, debug_taps=False,
                   do_node=True, do_pool=True, do_graph=True, wdma='gpsimd'):
    NBLK = S0 + S1
    assert NBLK % 2 == 0 and S0 % 2 == 0
    GP = 16 * NBLK
    NP = 512 * NBLK
    groups = []                          # (first_block, n_blocks<=4)
    b0 = 0
    while b0 < NBLK:
        gn = min(4, NBLK - b0)
        groups.append((b0, gn))
        b0 += gn
    NG = len(groups)

    nc = bacc.Bacc("TRN2", target_bir_lowering=False, debug=False,
                   num_devices=NCORES)

    xk_d = [nc.dram_tensor(f"xk{k}", [NP, 128], bf16, kind="ExternalInput").ap()
            for k in range(2)]
    wbf_d = nc.dram_tensor("wbf", [128, WBF_COLS], bf16, kind="ExternalInput").ap()
    wf32_d = nc.dram_tensor("wf32", [128, WF32_COLS], f32, kind="ExternalInput").ap()
    wgs_d = nc.dram_tensor("wgs", [128, WGS_COLS], f32r, kind="ExternalInput").ap()
    nraw_d = nc.dram_tensor("nraw", [128, NG * 512], f32, kind="ExternalOutput").ap()
    nsq_d = nc.dram_tensor("nsq", [128, NG * 512], f32, kind="ExternalOutput").ap()
    gT_d = nc.dram_tensor("gT", [128, GP], f32, kind="ExternalOutput").ap()
    dbg_d = {}
    if debug_taps:
        dbg_d["xt00"] = nc.dram_tensor("dbg_xt00", [128, 3072], f32,
                                       kind="ExternalOutput").ap()
        dbg_d["hnT0"] = nc.dram_tensor("dbg_hnT0", [128, 1024], f32,
                                      kind="ExternalOutput").ap()
        dbg_d["xgT0"] = nc.dram_tensor("dbg_xgT0", [128, GP], f32,
                                      kind="ExternalOutput").ap()
        dbg_d["xt00e"] = nc.dram_tensor("dbg_xt00e", [128, 3072], f32,
                                        kind="ExternalOutput").ap()

    # x slices for transpose loads: 6 blocks each (even, for super-blocks)
    slices = []
    b0 = 0
    while b0 < NBLK:
        sn = min(6, NBLK - b0)
        slices.append((b0, sn))
        b0 += sn
    sb_slice = {}  # block -> (slice idx, local block idx)
    for si, (sb0, sn) in enumerate(slices):
        for b in range(sb0, sb0 + sn):
            sb_slice[b] = (si, b - sb0)

    with tile.TileContext(nc) as tc:
        with (
            tc.tile_pool(name="wp", bufs=1) as wp,
            tc.tile_pool(name="xp", bufs=1) as xp,
            tc.tile_pool(name="fp", bufs=1) as fp,
            tc.tile_pool(name="hp", bufs=3) as hp,
            tc.tile_pool(name="op", bufs=2) as op,
            tc.tile_pool(name="psm", bufs=3, space="PSUM") as psm,
            tc.tile_pool(name="pss", bufs=2, space="PSUM") as pss,
        ):
            # ---------- weights: 3 packed DMAs on the scalar ring ----------
            wbf = wp.tile([128, WBF_COLS], bf16, name="wbf")
            wf = wp.tile([128, WF32_COLS], f32, name="wf")
            wgs = wp.tile([128, WGS_COLS], f32r, name="wgs")
            _weng = {"sync": nc.sync, "gpsimd": nc.gpsimd, "scalar": nc.scalar}[wdma]
            _weng.dma_start(out=wbf, in_=wbf_d)
            _weng.dma_start(out=wf, in_=wf32_d)
            _weng.dma_start(out=wgs, in_=wgs_d)

            def wn1(b, k, m):          # lhsT [128, 128] for L1 m-chunk
                c = (b * 2 + k) * 256
                return wbf[:, c + m * 128: c + (m + 1) * 128]

            def wn2(b, k):             # lhsT [128, 32] (cols 6.. zero)
                c = 1024 + (b * 2 + k) * 32
                return wbf[:, c: c + 32]

            def wgh(b, k):             # lhsT [128, 128] (head 0:50, var 64:114)
                c = 1152 + (b * 2 + k) * 128
                return wbf[:, c: c + 128]

            def wgsv(b, k, m):         # lhsT [128, 128] f32r
                c = (b * 2 + k) * 256
                return wgs[:, c + m * 128: c + (m + 1) * 128]

            def bn1(b, m):
                return wf[:, b * 2 + m: b * 2 + m + 1]

            def bgs(b, m):
                return wf[:, 4 + b * 2 + m: 5 + b * 2 + m]

            def bn2col(b):
                return wf[:, 8 + b: 9 + b]

            def bghT(b):
                return wf[:, 10 + b: 11 + b]

            for _rep in range(repeat):
                # ---------- x transposed loads (alternate HWDGE rings) ----------
                xT = [[xp.tile([128, 512 * sn], bf16, name=f"xT_{k}_{si}",
                               tag=f"xT_{k}_{si}")
                       for si, (sb0, sn) in enumerate(slices)] for k in range(2)]
                for si, (sb0, sn) in enumerate(slices):
                    for k in range(2):
                        nc.sync.dma_start_transpose(
                            out=xT[k][si],
                            in_=xk_d[k][sb0 * 512:(sb0 + sn) * 512, :])
                if debug_taps:
                    dxe = op.tile([128, 3072], f32, name="dxe", tag="dxe", bufs=1)
                    nc.vector.tensor_copy(dxe, xT[0][0])
                    nc.gpsimd.dma_start(out=dbg_d["xt00e"], in_=dxe)

                # f2 accumulation buffers (whole-GP, written per slice)
                f2b = [fp.tile([128, GP, 8], bf16, name=f"f2b_{k}", tag=f"f2b_{k}")
                       for k in range(2)] if do_pool else []
                xgT = [fp.tile([128, GP], f32r, name=f"xgT_{k}", tag=f"xgT_{k}")
                       for k in range(2)] if do_pool else []

                relu_state = [0]
                nrawb = op.tile([128, NG * 512], f32, name="nrawb", tag="nrawb",
                                bufs=1)
                nsqb = op.tile([128, NG * 512], f32, name="nsqb", tag="nsqb",
                               bufs=1)
                if NBLK % 4:
                    nc.vector.memset(nrawb[64:128, (NG - 1) * 512:], 0.0)
                    nc.vector.memset(nsqb[64:128, (NG - 1) * 512:], 0.0)

                def fold12(k, si):
                    sb0, sn = slices[si]
                    gn = sn * 16
                    v = xT[k][si].rearrange("p (g s) -> p g s", s=32)
                    f1 = fp.tile([128, gn, 16], bf16, name="f1", tag="f1")
                    nc.vector.tensor_add(f1, v[:, :, 0:16], v[:, :, 16:32])
                    nc.vector.tensor_add(f2b[k][:, sb0 * 16: sb0 * 16 + gn, :],
                                         f1[:, :, 0:8], f1[:, :, 8:16])

                def node_super_block(sb):
                    blkA = sb * 2
                    bA = 0 if blkA < S0 else 1
                    bB = 0 if blkA + 1 < S0 else 1
                    assert bA == bB  # S0 even => no straddling super-block
                    si, lb = sb_slice[blkA]
                    xv = [[xT[k][si][:, (lb + h) * 512:(lb + h + 1) * 512]
                           for h in range(2)] for k in range(2)]
                    hnT = []
                    for m in range(2):
                        ps = psm.tile([128, 1024], f32, name="mm", tag="mm")
                        for h in range(2):
                            for k in range(2):
                                nc.tensor.matmul(
                                    ps[:, h * 512:(h + 1) * 512],
                                    wn1(bA, k, m), xv[k][h],
                                    start=(k == 0), stop=(k == 1))
                        if debug_taps and sb == 0 and m == 0:
                            dps = op.tile([128, 1024], f32, name="dps", tag="dps",
                                          bufs=1)
                            nc.vector.tensor_copy(dps, ps)
                            nc.gpsimd.dma_start(out=dbg_d["hnT0"], in_=dps)
                        h_t = hp.tile([128, 1024], bf16, name="hnT", tag="hnT")
                        if relu_state[0] % relu_dve_mod == relu_dve_mod - 1:
                            nc.vector.tensor_scalar(
                                h_t, ps, bn1(bA, m), 0.0,
                                op0=ALU.add, op1=ALU.max)
                        else:
                            nc.scalar.activation(h_t, ps, AF.Relu, bias=bn1(bA, m))
                        relu_state[0] += 1
                        hnT.append(h_t)
                    return hnT

                def l2_group(gi, hnTs):
                    blk0, gn = groups[gi]
                    ps = pss.tile([128, 512], f32, name="og", tag="og")
                    for j in range(gn):
                        bb = 0 if blk0 + j < S0 else 1
                        for k in range(2):
                            hs = hnTs[j // 2][k][:, (j % 2) * 512:(j % 2 + 1) * 512]
                            nc.tensor.matmul(
                                ps[32 * j: 32 * j + 32, :], wn2(bb, k), hs,
                                start=(k == 0), stop=(k == 1),
                                tile_position=(0, 32 * j))
                    prt = 32 * (gn - 1) + 32
                    raw = nrawb[:, gi * 512:(gi + 1) * 512]
                    sq = nsqb[:, gi * 512:(gi + 1) * 512]
                    b0g = 0 if blk0 < S0 else 1
                    b1g = 0 if blk0 + gn - 1 < S0 else 1
                    if b0g == b1g:
                        nc.scalar.activation(raw[:prt], ps[:prt], AF.Identity,
                                             bias=bn2col(b0g)[:prt])
                        nc.scalar.activation(sq[:prt], ps[:prt], AF.Square,
                                             bias=bn2col(b0g)[:prt])
                    else:
                        split = 32 * (S0 - blk0)
                        for (p0, p1, bb) in ((0, split, b0g), (split, prt, b1g)):
                            nc.scalar.activation(raw[p0:p1], ps[p0:p1], AF.Identity,
                                                 bias=bn2col(bb)[p0:p1])
                            nc.scalar.activation(sq[p0:p1], ps[p0:p1], AF.Square,
                                                 bias=bn2col(bb)[p0:p1])


                # ---------- main loop: folds + node path, slice-pipelined ----------
                pend_hnT = {}
                for si in range(len(slices)):
                    for k in range(2) if do_pool else []:
                        fold12(k, si)
                    sb0, sn = slices[si]
                    for sb in (range(sb0 // 2, (sb0 + sn) // 2) if do_node else []):
                        pend_hnT[sb] = node_super_block(sb)
                        gi = (sb * 2) // 4
                        blk0, gn = groups[gi]
                        if (sb * 2 + 2) == blk0 + gn:  # group complete
                            first = pend_hnT.pop(blk0 // 2)
                            second = pend_hnT.pop(blk0 // 2 + 1, None)
                            hnTs = [first, second if second is not None else first]
                            l2_group(gi, hnTs)

                # ---------- finish pooling: f3..f5 whole-GP on DVE ----------
                for k in range(2) if do_pool else []:
                    f3 = fp.tile([128, GP, 4], f32, name="f3", tag="f3")
                    nc.vector.tensor_add(f3, f2b[k][:, :, 0:4], f2b[k][:, :, 4:8])
                    f4 = fp.tile([128, GP, 2], f32, name="f4", tag="f4")
                    nc.vector.tensor_add(f4, f3[:, :, 0:2], f3[:, :, 2:4])
                    nc.vector.tensor_add(
                        xgT[k].rearrange("p (g o) -> p g o", o=1),
                        f4[:, :, 0:1], f4[:, :, 1:2])

                # ---------- graph head (transposed) ----------
                gbufT = op.tile([128, GP], f32, name="gbufT", tag="gbufT")
                nc.vector.memset(gbufT, 0.0)
                if not do_node:
                    nc.vector.memset(nrawb, 0.0)
                    nc.vector.memset(nsqb, 0.0)
                for sec, S in (((0, S0), (1, S1)) if (do_graph and do_pool) else ()):
                    gsz = 16 * S
                    gs0 = 0 if sec == 0 else 16 * S0
                    hgT = []
                    for m in range(2):
                        ps = psm.tile([128, 1024], f32, name="mm", tag="mm")
                        for k in range(2):
                            nc.tensor.matmul(ps[:, :gsz], wgsv(sec, k, m),
                                             xgT[k][:, gs0:gs0 + gsz],
                                             start=(k == 0), stop=(k == 1))
                        h_t = hp.tile([128, 1024], bf16, name="hgT", tag="hgT",
                                      bufs=4)
                        nc.scalar.activation(h_t[:, :gsz], ps[:, :gsz], AF.Relu,
                                             bias=bgs(sec, m))
                        hgT.append(h_t)
                    ps = pss.tile([128, 512], f32, name="og", tag="og")
                    for k in range(2):
                        nc.tensor.matmul(ps[:, :gsz], wgh(sec, k),
                                         hgT[k][:, :gsz],
                                         start=(k == 0), stop=(k == 1))
                    nc.scalar.activation(gbufT[0:64, gs0:gs0 + gsz],
                                         ps[0:64, :gsz], AF.Identity,
                                         bias=bghT(sec)[0:64])
                    nc.scalar.activation(gbufT[64:128, gs0:gs0 + gsz],
                                         ps[64:128, :gsz], AF.Square,
                                         bias=bghT(sec)[64:128])
                if debug_taps:
                    dxt = op.tile([128, 3072], f32, name="dxt", tag="dxt", bufs=1)
                    nc.vector.tensor_copy(dxt, xT[0][0])
                    nc.gpsimd.dma_start(out=dbg_d["xt00"], in_=dxt)
                    if do_pool:
                        dxg = op.tile([128, GP], f32, name="dxg", tag="dxg", bufs=1)
                        nc.vector.tensor_copy(dxg, xgT[0])
                        nc.gpsimd.dma_start(out=dbg_d["xgT0"], in_=dxg)
                nc.gpsimd.dma_start(out=nraw_d, in_=nrawb)
                nc.gpsimd.dma_start(out=nsq_d, in_=nsqb)
                nc.gpsimd.dma_start(out=gT_d, in_=gbufT)

    nc.compile()
    return nc


def _prepare(x, dataset_name, Wgs, bgs, Wgh, bgh, Wn1, bn1, Wn2, bn2):
    ds = np.asarray(dataset_name)
    assert ds.shape == (G,)
    g0 = np.where(ds == 0)[0]
    g1 = np.where(ds != 0)[0]
    per0 = [g0[c::NCORES] for c in range(NCORES)]
    per1 = [g1[c::NCORES] for c in range(NCORES)]
    S0 = max(1, math.ceil(max(len(p) for p in per0) / 16))
    S1 = max(1, math.ceil(max(len(p) for p in per1) / 16))
    if S0 % 2:
        S0 += 1                    # super-blocks must stay branch-uniform
    if (S0 + S1) % 2:
        S1 += 1
    NBLK = S0 + S1
    GP = 16 * NBLK

    slot = np.zeros((NCORES, GP), dtype=np.int64)
    valid = np.zeros((NCORES, GP), dtype=bool)
    for c in range(NCORES):
        s0 = np.full(16 * S0, per0[c][0] if len(per0[c]) else 0, dtype=np.int64)
        s0[: len(per0[c])] = per0[c]
        s1 = np.full(16 * S1, per1[c][0] if len(per1[c]) else 0, dtype=np.int64)
        s1[: len(per1[c])] = per1[c]
        slot[c] = np.concatenate([s0, s1])
        valid[c, : len(per0[c])] = True
        valid[c, 16 * S0: 16 * S0 + len(per1[c])] = True

    xb = np.asarray(x).astype(ml_dtypes.bfloat16).reshape(G, NPG, D)

    wbf = np.zeros((128, WBF_COLS), ml_dtypes.bfloat16)
    wn1b = np.asarray(Wn1).astype(ml_dtypes.bfloat16)
    wn2b = np.asarray(Wn2).astype(ml_dtypes.bfloat16)
    wghb = np.asarray(Wgh).astype(ml_dtypes.bfloat16)
    for b in range(2):
        for k in range(2):
            i = b * 2 + k
            wbf[:, i * 256:(i + 1) * 256] = wn1b[b, k * 128:(k + 1) * 128, :]
            wbf[:, 1024 + i * 32: 1024 + i * 32 + 6] = wn2b[b, k * 128:(k + 1) * 128, :]
            wbf[:, 1152 + i * 128: 1152 + i * 128 + 50] = wghb[b, k * 128:(k + 1) * 128, :50]
            wbf[:, 1152 + i * 128 + 64: 1152 + i * 128 + 114] = wghb[b, k * 128:(k + 1) * 128, 50:]

    wf = np.zeros((128, WF32_COLS), np.float32)
    bn1f = np.asarray(bn1, np.float32)
    bgsf = np.asarray(bgs, np.float32)
    bn2f = np.asarray(bn2, np.float32)
    bghf = np.asarray(bgh, np.float32)
    for b in range(2):
        for m in range(2):
            wf[:, b * 2 + m] = bn1f[b, m * 128:(m + 1) * 128]
            wf[:, 4 + b * 2 + m] = bgsf[b, m * 128:(m + 1) * 128]
        col = np.zeros(128, np.float32)
        for j in range(4):
            col[32 * j: 32 * j + 6] = bn2f[b]
        wf[:, 8 + b] = col
        colg = np.zeros(128, np.float32)
        colg[:50] = bghf[b, :50]
        colg[64:114] = bghf[b, 50:]
        wf[:, 10 + b] = colg

    wgsp = np.zeros((128, WGS_COLS), np.float32)
    wgs_s = np.asarray(Wgs, np.float32) / NPG
    for b in range(2):
        for k in range(2):
            i = b * 2 + k
            wgsp[:, i * 256:(i + 1) * 256] = wgs_s[b, k * 128:(k + 1) * 128, :]

    shared = {"wbf": wbf, "wf32": wf, "wgs": wgsp}
    in_maps = []
    for c in range(NCORES):
        xc = xb[slot[c]].reshape(GP * NPG, D)
        m = {f"xk{k}": np.ascontiguousarray(xc[:, k * 128:(k + 1) * 128])
             for k in range(2)}
        m.update(shared)
        in_maps.append(m)
    return S0, S1, slot, valid, in_maps


def _unpack(results, S0, S1, slot, valid):
    NBLK = S0 + S1
    GP = 16 * NBLK

    n_head = np.zeros((N, HDN), np.float32)
    n_var = np.zeros((N, HDN), np.float32)
    g_head = np.zeros((G, HDG), np.float32)
    g_var = np.zeros((G, HDG), np.float32)

    for c in range(NCORES):
        rawf = results[c]["nraw"]   # [128, NG*512]
        NG = rawf.shape[1] // 512
        raw = rawf.reshape(128, NG, 512).transpose(1, 0, 2)
        sq = results[c]["nsq"].reshape(128, NG, 512).transpose(1, 0, 2)
        nh = np.empty((NBLK, 512, HDN), np.float32)
        nv = np.empty((NBLK, 512, HDN), np.float32)
        for gi in range(NG):
            gn = min(4, NBLK - 4 * gi)
            for j in range(gn):
                blk = 4 * gi + j
                nh[blk] = raw[gi, 32 * j: 32 * j + 3, :].T
                nv[blk] = sq[gi, 32 * j + 3: 32 * j + 6, :].T
        nh = nh.reshape(NBLK * 512, HDN)
        nv = nv.reshape(NBLK * 512, HDN)

        v = valid[c]
        gids = slot[c][v]
        sl = np.where(v)[0]
        nodes = (gids[:, None] * NPG + np.arange(NPG)[None, :]).reshape(-1)
        src = (sl[:, None] * NPG + np.arange(NPG)[None, :]).reshape(-1)
        n_head[nodes] = nh[src]
        n_var[nodes] = nv[src]

        gT = results[c]["gT"]       # [128, GP]
        g_head[gids] = gT[:50, sl].T
        g_var[gids] = gT[64:114, sl].T
    return g_head, n_head, g_var, n_var


def kernel(x, dataset_name, batch, Wgs, bgs, Wgh, bgh, Wn1, bn1, Wn2, bn2):
    S0, S1, slot, valid, in_maps = _prepare(
        x, dataset_name, Wgs, bgs, Wgh, bgh, Wn1, bn1, Wn2, bn2)
    key = (S0, S1)
    if key not in _programs:
        _programs[key] = _build_program(S0, S1)
    nc = _programs[key]
    res = bass_utils.run_bass_kernel_spmd(nc, in_maps,
                                          core_ids=list(range(NCORES)))
    return _unpack(res.results, S0, S1, slot, valid)
